# revision 15
# baseline (speedup 1.0000x reference)
# Trainium2 Bass kernel for nn_ConditionalVariationalModule_75299366633595.
#
# Reference computation (see problem spec): a conditional VAE scan over
# S=256 timesteps. Per step t (batch B=1024):
#   prior_out = MLP3([h_t, z], pW*)          -> pm, plv      (2*128)
#   post_out  = MLP3([h_t, z, h_t], qW*)     -> qm, qlv
#   z_t = qm + eps_t * exp(0.5*qlv)
# Outputs: z, pm, plv, qm, qlv each [B, S, 128] (returned as a tuple).
#
# Strategy (8 cores, data-parallel over batch, 128 samples/core):
# - Everything is kept feature-major on device ([feature, batch]) so matmul
#   outputs feed the next layer with no transposes. Host does all transposes.
# - Posterior is the sequential recurrence. Per step, only the z-dependent
#   part of layer 1 is on the critical path; the h-part and biases are
#   accumulated into PSUM early (identity-matmul bias trick).
# - z is never materialized on the critical path: the next step's layer-1
#   matmuls take qm (bf16) and prod = eps*exp(0.5qlv+0.5b) (bf16) as two
#   separate rhs operands (W.T@(qm+prod) == W.T@qm + W.T@prod).
# - The prior MLP never feeds the recurrence, so it runs as batched N=512
#   GEMMs over 4-step token blocks, reading a bf16 z history from DRAM.
# - Matmuls in bf16 (fp32 matmul is 4x slower on TRN2 PE), accumulation fp32.

import os
import numpy as np
import ml_dtypes

import concourse.bass as bass
import concourse.mybir as mybir
import concourse.tile as tile
from concourse import bacc
from concourse import bass_utils

AF = mybir.ActivationFunctionType
F32 = mybir.dt.float32
BF16 = mybir.dt.bfloat16

NCORES = 8
B_TOTAL = 1024
BC = B_TOTAL // NCORES  # 128 batch per core
S = 256
D = 256  # input dim
L = 128  # latent dim
H = 256  # hidden dim


# --------------------------------------------------------------------------
# Device program
# --------------------------------------------------------------------------

def build_nc(n_steps=S, interleave=True):
    """Build the per-core Bass program (SPMD across 8 cores)."""
    nc = bacc.Bacc("TRN2", target_bir_lowering=False, debug=False,
                   num_devices=NCORES)

    # ---- DRAM I/O ----
    hT = nc.dram_tensor("hT", [2, 128, n_steps * BC], BF16, kind="ExternalInput")
    epsT = nc.dram_tensor("epsT", [128, n_steps * BC], BF16, kind="ExternalInput")
    z0T = nc.dram_tensor("z0T", [128, BC], BF16, kind="ExternalInput")

    wspec = {
        "qW1h": [2, 128, 256], "qW1z": [128, 256],
        "qW2": [2, 128, 256], "qW3": [2, 128, 256],
        "pW1h": [2, 128, 256], "pW1z": [128, 256],
        "pW2": [2, 128, 256], "pW3": [2, 128, 256],
        "B1q": [128, 256], "B2q": [128, 256],
        "ident": [128, 128],
    }
    wdram = {k: nc.dram_tensor(k, shp, BF16, kind="ExternalInput")
             for k, shp in wspec.items()}
    bspec = {
        "qb3m": [128, 1], "qb3lv": [128, 1], "qb3lvh": [128, 1],
        "pb1c": [128, 2], "pb2c": [128, 2],
        "pb3m": [128, 1], "pb3lv": [128, 1],
    }
    bdram = {k: nc.dram_tensor(k, shp, F32, kind="ExternalInput")
             for k, shp in bspec.items()}

    outs = {k: nc.dram_tensor(k, [n_steps, 128, BC], F32, kind="ExternalOutput")
            for k in ("z_out", "qm_out", "qlv_out", "pm_out", "plv_out")}

    n_blocks = n_steps // 4
    assert n_steps % 4 == 0

    with tile.TileContext(nc) as tc:
        with (
            tc.tile_pool(name="const", bufs=1) as const,
            tc.tile_pool(name="dram", bufs=1, space="DRAM") as dpool,
            tc.tile_pool(name="hp", bufs=6) as hpool,
            tc.tile_pool(name="ep", bufs=3) as epool,
            tc.tile_pool(name="sp", bufs=3) as spool,
            tc.tile_pool(name="blk", bufs=3) as bpool,
            tc.tile_pool(name="p3", bufs=3) as p3pool,
            tc.tile_pool(name="ps", bufs=2, space="PSUM") as psp,
        ):
            # ---- constants into SBUF ----
            w = {}
            for k in ("qW1h", "qW2", "qW3", "pW1h", "pW2", "pW3"):
                t_ = const.tile([128, 2, 256], BF16, tag=k)
                nc.sync.dma_start(t_[:], wdram[k].ap().rearrange("k d h -> d k h"))
                w[k] = t_
            for k in ("qW1z", "pW1z", "B1q", "B2q"):
                t_ = const.tile([128, 256], BF16, tag=k)
                nc.sync.dma_start(t_[:], wdram[k].ap())
                w[k] = t_
            ident = const.tile([128, 128], BF16, tag="ident")
            nc.sync.dma_start(ident[:], wdram["ident"].ap())
            z0t = const.tile([128, BC], BF16, tag="z0T")
            nc.sync.dma_start(z0t[:], z0T.ap())
            bias = {}
            for k, shp in bspec.items():
                t_ = const.tile(shp, F32, tag=k)
                nc.sync.dma_start(t_[:], bdram[k].ap())
                bias[k] = t_

            # bf16 z history in DRAM: slot s holds z_{s-1} (slot 0 = z0)
            zhist = dpool.tile([128, (n_steps + 1) * BC], BF16)
            nc.sync.dma_start(zhist[:, 0:BC], z0T.ap())

            # ---- streaming input tiles (4 steps per group) ----
            htiles, etiles = {}, {}

            def load_group(g):
                if g < 0 or g * 4 >= n_steps or g in htiles:
                    return
                ht = hpool.tile([128, 2, 4 * BC], BF16, tag="h")
                nc.sync.dma_start(
                    ht[:], hT.ap()[:, :, g * 4 * BC:(g + 1) * 4 * BC]
                    .rearrange("k d f -> d k f"))
                et = epool.tile([128, 4 * BC], BF16, tag="e")
                nc.sync.dma_start(et[:], epsT.ap()[:, g * 4 * BC:(g + 1) * 4 * BC])
                htiles[g] = ht
                etiles[g] = et

            load_group(0)
            load_group(1)

            qm_prev = prod_prev = None
            cur_qmblk = cur_prodblk = None

            HC = [(0, slice(0, 128)), (1, slice(128, 256))]

            def emit_prior_block(j):
                """Prior MLP for tokens [4j, 4j+4) — N=512 batched GEMMs."""
                zt_ = p3pool.tile([128, 4 * BC], BF16, tag="zblk")
                nc.sync.dma_start(zt_[:], zhist[:, j * 4 * BC:(j + 1) * 4 * BC])
                ht2 = hpool.tile([128, 2, 4 * BC], BF16, tag="h")
                nc.sync.dma_start(
                    ht2[:], hT.ap()[:, :, j * 4 * BC:(j + 1) * 4 * BC]
                    .rearrange("k d f -> d k f"))

                ps1 = [psp.tile([128, 4 * BC], F32, tag="p3ps", bufs=3, name=f"ps1_{hc}") for hc in (0, 1)]
                for hc, hs in HC:
                    nc.tensor.matmul(ps1[hc][:], w["pW1h"][:, 0, hs], ht2[:, 0, :],
                                     start=True, stop=False)
                    nc.tensor.matmul(ps1[hc][:], w["pW1h"][:, 1, hs], ht2[:, 1, :],
                                     start=False, stop=False)
                    nc.tensor.matmul(ps1[hc][:], w["pW1z"][:, hs], zt_[:],
                                     start=False, stop=True)
                h1p = p3pool.tile([128, 2, 4 * BC], BF16, tag="h1p")
                nc.scalar.activation(h1p[:, 0, :], ps1[0][:], AF.Relu,
                                     bias=bias["pb1c"][:, 0:1])
                nc.vector.tensor_scalar(h1p[:, 1, :], ps1[1][:],
                                        bias["pb1c"][:, 1:2], 0.0,
                                        mybir.AluOpType.add, mybir.AluOpType.max)
                ps2 = [psp.tile([128, 4 * BC], F32, tag="p3ps", bufs=3, name=f"ps2_{hc}") for hc in (0, 1)]
                for hc, hs in HC:
                    for kc in (0, 1):
                        nc.tensor.matmul(ps2[hc][:], w["pW2"][:, kc, hs],
                                         h1p[:, kc, :],
                                         start=(kc == 0), stop=(kc == 1))
                h2p = p3pool.tile([128, 2, 4 * BC], BF16, tag="h2p")
                nc.scalar.activation(h2p[:, 0, :], ps2[0][:], AF.Relu,
                                     bias=bias["pb2c"][:, 0:1])
                nc.vector.tensor_scalar(h2p[:, 1, :], ps2[1][:],
                                        bias["pb2c"][:, 1:2], 0.0,
                                        mybir.AluOpType.add, mybir.AluOpType.max)
                ps3 = [psp.tile([128, 4 * BC], F32, tag="p3ps", bufs=3, name=f"ps3_{hc}") for hc in (0, 1)]
                for hc, hs in HC:
                    for kc in (0, 1):
                        nc.tensor.matmul(ps3[hc][:], w["pW3"][:, kc, hs],
                                         h2p[:, kc, :],
                                         start=(kc == 0), stop=(kc == 1))
                pm_sb = p3pool.tile([128, 4 * BC], F32, tag="pm_sb")
                nc.vector.tensor_scalar_add(pm_sb[:], ps3[0][:], bias["pb3m"][:, 0:1])
                plv_sb = p3pool.tile([128, 4 * BC], F32, tag="plv_sb")
                nc.vector.tensor_scalar_add(plv_sb[:], ps3[1][:], bias["pb3lv"][:, 0:1])
                nc.sync.dma_start(
                    outs["pm_out"].ap()[j * 4:(j + 1) * 4].rearrange("t l b -> l t b"),
                    pm_sb[:].rearrange("l (t b) -> l t b", b=BC))
                nc.sync.dma_start(
                    outs["plv_out"].ap()[j * 4:(j + 1) * 4].rearrange("t l b -> l t b"),
                    plv_sb[:].rearrange("l (t b) -> l t b", b=BC))

            # ================= the scan =================
            for t in range(n_steps):
                g, sl = t // 4, t % 4
                if sl == 0:
                    load_group(g + 2)
                ht, et = htiles[g], etiles[g]
                bsl = slice(sl * BC, (sl + 1) * BC)

                # ---- layer 1 (posterior): bias + h-part early, z-part last
                psum1 = psp.tile([128, 256], F32, tag="l1", bufs=1)
                nc.tensor.matmul(psum1[:], ident[:], w["B1q"][:],
                                 start=True, stop=False)
                for hc, hs in HC:
                    for kc in (0, 1):
                        nc.tensor.matmul(psum1[:, hs], w["qW1h"][:, kc, hs],
                                         ht[:, kc, bsl], start=False, stop=False)
                for hc, hs in HC:
                    last = hc == 1
                    if t == 0:
                        nc.tensor.matmul(psum1[:, hs], w["qW1z"][:, hs], z0t[:],
                                         start=False, stop=last)
                    else:
                        nc.tensor.matmul(psum1[:, hs], w["qW1z"][:, hs], qm_prev,
                                         start=False, stop=False)
                        nc.tensor.matmul(psum1[:, hs], w["qW1z"][:, hs], prod_prev,
                                         start=False, stop=last)
                h1 = spool.tile([128, 256], BF16, tag="h1")
                nc.scalar.activation(h1[:], psum1[:], AF.Relu)

                # ---- layer 2
                psum2 = psp.tile([128, 256], F32, tag="l2", bufs=1)
                nc.tensor.matmul(psum2[:], ident[:], w["B2q"][:],
                                 start=True, stop=False)
                for hc, hs in HC:
                    for kc in (0, 1):
                        nc.tensor.matmul(psum2[:, hs], w["qW2"][:, kc, hs],
                                         h1[:, kc * 128:(kc + 1) * 128],
                                         start=False, stop=(hc == 1 and kc == 1))
                h2 = spool.tile([128, 256], BF16, tag="h2")
                nc.scalar.activation(h2[:], psum2[:], AF.Relu)

                # ---- layer 3 -> [qm | qlv]
                psum3 = psp.tile([128, 256], F32, tag="l3", bufs=2)
                for hc, hs in HC:
                    for kc in (0, 1):
                        nc.tensor.matmul(psum3[:, hs], w["qW3"][:, kc, hs],
                                         h2[:, kc * 128:(kc + 1) * 128],
                                         start=(hc == 0 and kc == 0),
                                         stop=(hc == 1 and kc == 1))

                # ---- recurrence + outputs
                ehalf = spool.tile([128, 128], F32, tag="eh")
                nc.scalar.activation(ehalf[:], psum3[:, 128:256], AF.Exp,
                                     bias=bias["qb3lvh"][:, 0:1], scale=0.5)

                if t + 1 < n_steps:
                    if (t + 1) % 4 == 0 or cur_qmblk is None:
                        cur_qmblk = bpool.tile([128, 4, BC], BF16, tag="qmb")
                        cur_prodblk = bpool.tile([128, 4, BC], BF16, tag="prb")
                    ws = (t + 1) % 4
                    qm_prev = cur_qmblk[:, ws, :]
                    prod_prev = cur_prodblk[:, ws, :]
                    nc.vector.tensor_scalar_add(qm_prev, psum3[:, 0:128],
                                                bias["qb3m"][:, 0:1])
                    nc.vector.tensor_mul(prod_prev, ehalf[:], et[:, bsl])

                if sl == 0:
                    ob_qm = spool.tile([128, 4, BC], F32, tag="ob_qm")
                    ob_qlv = spool.tile([128, 4, BC], F32, tag="ob_qlv")
                    ob_z = spool.tile([128, 4, BC], F32, tag="ob_z")
                    ob_zbf = spool.tile([128, 4, BC], BF16, tag="ob_zbf")
                qm_f = ob_qm[:, sl, :]
                nc.vector.tensor_scalar_add(qm_f, psum3[:, 0:128],
                                            bias["qb3m"][:, 0:1])
                nc.vector.tensor_scalar_add(ob_qlv[:, sl, :], psum3[:, 128:256],
                                            bias["qb3lv"][:, 0:1])
                prod_f = spool.tile([128, 128], F32, tag="prodf")
                nc.gpsimd.tensor_mul(prod_f[:], ehalf[:], et[:, bsl])
                nc.gpsimd.tensor_add(ob_z[:, sl, :], qm_f, prod_f[:])
                nc.gpsimd.tensor_copy(ob_zbf[:, sl, :], ob_z[:, sl, :])
                if sl == 3:
                    g4 = slice(g * 4, g * 4 + 4)
                    nc.sync.dma_start(
                        outs["z_out"].ap()[g4].rearrange("t l b -> l t b"), ob_z[:])
                    nc.sync.dma_start(
                        outs["qm_out"].ap()[g4].rearrange("t l b -> l t b"), ob_qm[:])
                    nc.sync.dma_start(
                        outs["qlv_out"].ap()[g4].rearrange("t l b -> l t b"), ob_qlv[:])
                    nc.sync.dma_start(
                        zhist[:, (g * 4 + 1) * BC:(g * 4 + 5) * BC], ob_zbf[:])

                # interleave prior blocks into the scan's engine gaps,
                # lagged 2 blocks so the z-history DMA round-trip never
                # head-of-line-blocks the PE queue
                if interleave and t % 4 == 3 and t >= 11:
                    emit_prior_block((t - 3) // 4 - 2)

            if interleave:
                emit_prior_block(n_blocks - 2)
                emit_prior_block(n_blocks - 1)
            else:
                for j in range(n_blocks):
                    emit_prior_block(j)

    nc.compile()
    return nc


# --------------------------------------------------------------------------
# Host-side data prep
# --------------------------------------------------------------------------

def prep_inputs(encoder_features, prev_latent, eps,
                pW1, pb1, pW2, pb2, pW3, pb3,
                qW1, qb1, qW2, qb2, qW3, qb3, n_steps=S):
    bf = ml_dtypes.bfloat16
    f32 = np.float32
    nco = NCORES

    enc = np.asarray(encoder_features, f32)[:, :n_steps]
    epsv = np.asarray(eps, f32)[:, :n_steps]
    prev = np.asarray(prev_latent, f32)

    # [core, kc, d, s, b]
    hT = np.ascontiguousarray(
        enc.reshape(nco, BC, n_steps, 2, 128).transpose(0, 3, 4, 2, 1)
    ).reshape(nco, 2, 128, n_steps * BC).astype(bf)
    epsT = np.ascontiguousarray(
        epsv.reshape(nco, BC, n_steps, 128).transpose(0, 3, 2, 1)
    ).reshape(nco, 128, n_steps * BC).astype(bf)
    z0T = np.ascontiguousarray(
        prev.reshape(nco, BC, 128).transpose(0, 2, 1)).astype(bf)

    def wchunks(wmat):  # [256, H'] -> [2, 128, H']
        return np.ascontiguousarray(np.asarray(wmat, f32).reshape(2, 128, -1)).astype(bf)

    qW1 = np.asarray(qW1, f32)
    pW1 = np.asarray(pW1, f32)
    shared = {
        "qW1h": wchunks(qW1[0:256] + qW1[384:640]),
        "qW1z": np.ascontiguousarray(qW1[256:384]).astype(bf),
        "qW2": wchunks(qW2), "qW3": wchunks(qW3),
        "pW1h": wchunks(pW1[0:256]),
        "pW1z": np.ascontiguousarray(pW1[256:384]).astype(bf),
        "pW2": wchunks(pW2), "pW3": wchunks(pW3),
        "B1q": np.ascontiguousarray(np.broadcast_to(
            np.asarray(qb1, f32).reshape(2, 128).T[:, :, None],
            (128, 2, BC))).reshape(128, 256).astype(bf),
        "B2q": np.ascontiguousarray(np.broadcast_to(
            np.asarray(qb2, f32).reshape(2, 128).T[:, :, None],
            (128, 2, BC))).reshape(128, 256).astype(bf),
        "ident": np.eye(128, dtype=f32).astype(bf),
        "qb3m": np.asarray(qb3, f32)[0:128].reshape(128, 1).copy(),
        "qb3lv": np.asarray(qb3, f32)[128:256].reshape(128, 1).copy(),
        "qb3lvh": (0.5 * np.asarray(qb3, f32)[128:256]).reshape(128, 1).copy(),
        "pb1c": np.ascontiguousarray(np.asarray(pb1, f32).reshape(2, 128).T),
        "pb2c": np.ascontiguousarray(np.asarray(pb2, f32).reshape(2, 128).T),
        "pb3m": np.asarray(pb3, f32)[0:128].reshape(128, 1).copy(),
        "pb3lv": np.asarray(pb3, f32)[128:256].reshape(128, 1).copy(),
    }
    in_maps = []
    for c in range(nco):
        m = {"hT": hT[c], "epsT": epsT[c], "z0T": z0T[c]}
        m.update(shared)
        in_maps.append(m)
    return in_maps


def unshard(results, n_steps=S):
    """results: list of per-core dicts with [n_steps, 128(l), BC(b)] f32."""
    def full(name):
        # -> [B_TOTAL, n_steps, 128]
        per = [r[name].transpose(2, 0, 1) for r in results]
        return np.ascontiguousarray(np.concatenate(per, axis=0))
    return (full("z_out"), full("pm_out"), full("plv_out"),
            full("qm_out"), full("qlv_out"))


_NC_CACHE = {}


def get_nc(n_steps=S, interleave=True):
    key = (n_steps, interleave)
    if key not in _NC_CACHE:
        _NC_CACHE[key] = build_nc(n_steps, interleave)
    return _NC_CACHE[key]


def kernel(**inputs):
    in_maps = prep_inputs(**inputs)
    nc = get_nc(S)
    res = bass_utils.run_bass_kernel_spmd(
        nc, in_maps, core_ids=list(range(NCORES)), trace=False)
    return unshard(res.results)
